# revision 1
# baseline (speedup 1.0000x reference)
"""Causal self-attention (GQA, qk-RMS-norm, RoPE) Trainium2 Bass kernel.

Sharding (8 cores): batch (2) x kv-head-group (4).  Core c handles batch
b = c // 4 and kv head g = c % 4 (with its 4 query heads 4g..4g+3).
Each core computes a (T, D) partial of the output projection (Wproj input
dim is split across the 4 tensor-parallel cores); the host sums the 4
partials per batch element.

Device-side formulation (per core):
  - host passes x^T, [Wq|Wk|Wv]^T slice, Wproj^T slice, rope tables (all
    bf16 except rope tables/consts)
  - QKV proj:  psum[t_tile, 384] = sum_k xT_tile^T . wqkvT_tile  (bf16)
  - RoPE applied to raw q/k straight out of PSUM (rotation commutes with
    the rms-norm scaling); the q rms-norm scale is applied in SBUF (cast
    to bf16 on the way), the k rms-norm scale and the 1/sqrt(hd) score
    scale are folded into the per-partition `scale` operand of the Exp.
  - scores are computed transposed (S^T[tk, tq]) so no softmax-max pass is
    needed (|score| <= 8 after rms norm) and A.V needs no transposes:
        S^T = kT^T . qT      (kT replicated into both 64-partition halves;
                              head pairs run concurrently via PE row groups)
        U = exp(0.125 * rstd_k[tk] * S^T)  (ScalarE, straight from PSUM,
                                            cast to bf16)
        Y^T[pair] += ([V|0] / [0|V])^T . U  (heads packed 2-per-psum-bank
                                             via zero-padded V operands)
        den[pair] += block-ones^T . U       (denominators land broadcast
                                             across each head's 64 rows)
  - causal masking: tk-tiles strictly above the diagonal are skipped
    entirely (column slicing), the single diagonal 128x128 block of u is
    zeroed by a multiplicative {0,1} mask on DVE after the exp (so the
    PE -> ScalarE exp chain never waits on masking).
  - y normalized by the reciprocal of the broadcast denominators, written
    as yT (d' on partitions) which directly feeds the projection matmuls.
  - emission order: all qkv groups (pipelined through one 4-bank psum
    tile), then attention chunks with the projection of chunk c emitted
    right after chunk c, sharing the denominator psum slots so it overlaps
    chunk c+1's score/exp work.  Cost-model timeline: ~198 us/core.
"""

import os
import sys

import ml_dtypes
import numpy as np

for _p in ("/opt/trn_rl_repo", "/root/.axon_site/_ro/trn_rl_repo"):
    if _p not in sys.path and os.path.isdir(_p):
        sys.path.append(_p)

import concourse.bass as bass
import concourse.bacc as bacc_mod
import concourse.mybir as mybir
import concourse.tile as tile
from concourse.bass import ts
from concourse.bass_utils import run_bass_kernel_spmd

F32 = mybir.dt.float32
BF16 = mybir.dt.bfloat16

B, T, D = 2, 2048, 1024
H, HKV, HD = 16, 4, 64
NH = H // HKV            # 4 local q heads per core
P = 128
NT = T // P              # 16 t tiles
ND = D // P              # 8 d tiles
GT = 4                   # t-tiles per qkv group
NG = NT // GT            # 4 groups
CW = 512                 # tq chunk width
NCHUNK = T // CW         # 4
QKV_W = NH * HD + 2 * HD  # 384 = q(256) | k(64) | v(64)
EPS = float(np.finfo(np.float32).eps)
SCALE = float(HD) ** -0.5
ROPE_BASE = 10000.0

# consts layout (columns of the f32 "consts" input, [P, NCONST])
C_TRI = 0      # 0:256   additive causal mask for diagonal blocks, 2 copies
C_QG = 256     # 256:272 q_gain per head, replicated GT times ([P, GT, 4])
C_EPS = 272    # 272:273 eps
NCONST = 273

# blk layout (columns of the bf16 "blk" input, [P, 512])
#   0:256   [0 x64 | 1 x64 | 0 x128]: slices give [1s|0s] / [0s|1s] lhsT
#           for the broadcast denominator matmuls
#   256:384 identity (bf16) for PE transposes
#   384:640 multiplicative causal tri mask {0,1}, 2 copies
BLK_W = 640


def _build_bass(debug=False):
    nc = bacc_mod.Bacc(trn_type="TRN2")

    xT_d = nc.dram_tensor("xT", [D, T], BF16, kind="ExternalInput")
    wqkv_d = nc.dram_tensor("wqkvT", [D, QKV_W], BF16, kind="ExternalInput")
    wproj_d = nc.dram_tensor("wprojT", [NH * HD, D], BF16, kind="ExternalInput")
    cos_d = nc.dram_tensor("cosT", [T, NH + 1, HD // 2], F32, kind="ExternalInput")
    sin_d = nc.dram_tensor("sinT", [T, NH + 1, HD // 2], F32, kind="ExternalInput")
    consts_d = nc.dram_tensor("consts", [P, NCONST], F32, kind="ExternalInput")
    blk_d = nc.dram_tensor("blk", [P, BLK_W], BF16, kind="ExternalInput")
    out_d = nc.dram_tensor("outp", [T, D], F32, kind="ExternalOutput")
    if debug:
        dbg_qT = nc.dram_tensor("dbg_qT", [P, 2, T], BF16, kind="ExternalOutput")
        dbg_kT = nc.dram_tensor("dbg_kT", [P, T], BF16, kind="ExternalOutput")
        dbg_v = nc.dram_tensor("dbg_v", [P, NT, 192], BF16, kind="ExternalOutput")
        dbg_rk = nc.dram_tensor("dbg_rk", [P, NT], F32, kind="ExternalOutput")
        dbg_yT = nc.dram_tensor("dbg_yT", [P, 2, T], BF16, kind="ExternalOutput")
        dbg_u = nc.dram_tensor("dbg_u", [P, NH, CW], BF16, kind="ExternalOutput")
        dbg_dr = nc.dram_tensor(
            "dbg_dr", [NCHUNK, 2, P, CW], F32, kind="ExternalOutput"
        )
        dbg_y = nc.dram_tensor(
            "dbg_y", [NCHUNK, 2, P, CW], F32, kind="ExternalOutput"
        )

    with tile.TileContext(nc) as tc:
        _b = lambda k, d: int(os.environ.get(f"KB_{k}", str(d)))
        with (
            tc.tile_pool(name="singles", bufs=1) as singles,
            tc.tile_pool(name="xg", bufs=4) as xg_pool,
            tc.tile_pool(name="qk", bufs=_b("QK", 2)) as qk_pool,
            tc.tile_pool(name="stat", bufs=_b("ST", 2)) as stat_pool,
            tc.tile_pool(name="u", bufs=_b("U", 6)) as u_pool,
            tc.tile_pool(name="r", bufs=2) as r_pool,
            tc.tile_pool(name="ob", bufs=_b("OB", 2)) as ob_pool,
        ):
            # ---------------- persistent SBUF ----------------
            wqkv_sb = singles.tile([P, ND, QKV_W], BF16)
            wproj_sb = singles.tile([P, 2, D], BF16)
            cos_sb = singles.tile([P, NT, NH + 1, HD // 2], F32)
            sin_sb = singles.tile([P, NT, NH + 1, HD // 2], F32)
            consts_sb = singles.tile([P, NCONST], F32)
            blk_sb = singles.tile([P, BLK_W], BF16)
            # pair pr: head 2pr at partitions 0:64, head 2pr+1 at 64:128
            qT_sb = singles.tile([P, 2, T], BF16)
            kT_sb = singles.tile([P, T], BF16)      # replicated into both halves
            v_sb = singles.tile([P, NT, 192], BF16)  # cols 64:128 hold V
            yT_sb = singles.tile([P, 2, T], BF16)
            rstdk_sb = singles.tile([P, NT], F32)   # 0.125 * rstd_k per tk tile

            nc.sync.dma_start(
                out=wqkv_sb, in_=wqkv_d[:].rearrange("(po pi) f -> pi po f", pi=P)
            )
            nc.sync.dma_start(
                out=wproj_sb, in_=wproj_d[:].rearrange("(po pi) f -> pi po f", pi=P)
            )
            nc.sync.dma_start(
                out=cos_sb, in_=cos_d[:].rearrange("(nt p) h f -> p nt h f", p=P)
            )
            nc.sync.dma_start(
                out=sin_sb, in_=sin_d[:].rearrange("(nt p) h f -> p nt h f", p=P)
            )
            nc.sync.dma_start(out=consts_sb, in_=consts_d[:])
            nc.sync.dma_start(out=blk_sb, in_=blk_d[:])
            nc.gpsimd.memset(v_sb, 0.0)

            qg4 = consts_sb[:, C_QG : C_QG + 16].rearrange("p (g h) -> p g h", g=GT)
            eps_ap = consts_sb[:, C_EPS : C_EPS + 1]
            ident = blk_sb[:, 256:384]
            tri2b = blk_sb[:, 384:640].rearrange("p (j f) -> p j f", j=2)

            def den_lhsT(h):
                # [1s|0s] for even heads (denominator broadcast over psum
                # rows 0:64), [0s|1s] for odd heads (rows 64:128)
                return blk_sb[:, 64:192] if h % 2 == 0 else blk_sb[:, 0:128]

            def emit_group(g, qkv_ps_pool, tr_ps_pool, qkv_tag="qkv"):
                """QKV projection + rms stats + rope + transposes for the
                4 t-tiles of group g.  The qkv psum is split into two
                2-bank sub-tiles so it can share slots with the score
                tiles in mixed emission order."""
                xg_sb = xg_pool.tile([P, ND, GT * P], BF16, tag="xg",
                                     name=f"xg{g}")
                for ik in range(ND):
                    nc.sync.dma_start(
                        out=xg_sb[:, ik, :],
                        in_=xT_d[ts(ik, P), ts(g, GT * P)],
                    )
                nqk = GT * (NH + 1)  # 20
                qk_raw = qk_pool.tile([P, nqk, HD], F32, tag="qkraw")
                if qkv_tag == "s":
                    # attention-interleaved mode: two 2-bank sub-tiles that
                    # fit the score-tile slots
                    for half in range(2):
                        qkv_ps = qkv_ps_pool.tile(
                            [P, 2, 512], F32, tag="s", name=f"qkv{g}_{half}"
                        )
                        for jj in range(2):
                            j = 2 * half + jj
                            for ik in range(ND):
                                nc.tensor.matmul(
                                    qkv_ps[:, jj, 0:QKV_W],
                                    lhsT=xg_sb[:, ik, ts(j, P)],
                                    rhs=wqkv_sb[:, ik, :],
                                    start=(ik == 0),
                                    stop=(ik == ND - 1),
                                )
                        nc.vector.tensor_copy(
                            qk_raw[
                                :, 2 * half * (NH + 1) : (2 * half + 2) * (NH + 1), :
                            ].rearrange("p n x -> p (n x)"),
                            qkv_ps[:, :, 0 : (NH + 1) * HD],
                        )
                        nc.vector.tensor_copy(
                            v_sb[:, ts(2 * g + half, 2), 64:128],
                            qkv_ps[:, :, 320:384],
                        )
                elif _b("QKV", 1) > 1:
                    # deep-buffered 2-bank sub-tiles: matmuls of later
                    # sub-tiles run while DVE drains earlier ones
                    for half in range(2):
                        qkv_ps = qkv_ps_pool.tile(
                            [P, 2, 512], F32, tag="qkv", name=f"qkv{g}_{half}"
                        )
                        for jj in range(2):
                            j = 2 * half + jj
                            for ik in range(ND):
                                nc.tensor.matmul(
                                    qkv_ps[:, jj, 0:QKV_W],
                                    lhsT=xg_sb[:, ik, ts(j, P)],
                                    rhs=wqkv_sb[:, ik, :],
                                    start=(ik == 0),
                                    stop=(ik == ND - 1),
                                )
                        nc.vector.tensor_copy(
                            qk_raw[
                                :, 2 * half * (NH + 1) : (2 * half + 2) * (NH + 1), :
                            ].rearrange("p n x -> p (n x)"),
                            qkv_ps[:, :, 0 : (NH + 1) * HD],
                        )
                        nc.vector.tensor_copy(
                            v_sb[:, ts(2 * g + half, 2), 64:128],
                            qkv_ps[:, :, 320:384],
                        )
                else:
                    qkv_ps = qkv_ps_pool.tile(
                        [P, GT, 512], F32, tag=qkv_tag, name=f"qkv{g}"
                    )
                    for j in range(GT):
                        for ik in range(ND):
                            nc.tensor.matmul(
                                qkv_ps[:, j, 0:QKV_W],
                                lhsT=xg_sb[:, ik, ts(j, P)],
                                rhs=wqkv_sb[:, ik, :],
                                start=(ik == 0),
                                stop=(ik == ND - 1),
                            )
                    # copy raw q|k out of psum into a tightly-packed tile:
                    # the (tile, head) dims collapse to one 20-wide dim,
                    # keeping every rope/stat AP in walrus' 3-dim TT limit
                    nc.vector.tensor_copy(
                        qk_raw.rearrange("p n x -> p (n x)"),
                        qkv_ps[:, :, 0 : (NH + 1) * HD],
                    )
                    # copy V out right away: it is the last psum reader, so
                    # the next group's matmuls can recycle the bank sooner
                    nc.vector.tensor_copy(
                        v_sb[:, ts(g, GT), 64:128], qkv_ps[:, :, 320:384]
                    )

                # rms statistics on raw q, k
                sq = qk_pool.tile([P, nqk, HD], F32, tag="sq")
                qk_flat = qk_raw.rearrange("p n x -> p (n x)")
                nc.vector.tensor_mul(
                    sq.rearrange("p n x -> p (n x)"), qk_flat, qk_flat
                )
                ssq = stat_pool.tile([P, nqk], F32, tag="ssq")
                nc.vector.reduce_sum(ssq, sq, axis=mybir.AxisListType.X)
                nc.scalar.activation(
                    out=ssq,
                    in_=ssq,
                    func=mybir.ActivationFunctionType.Sqrt,
                    bias=eps_ap,
                    scale=1.0 / HD,
                )
                rstd = stat_pool.tile([P, nqk], F32, tag="rstd")
                nc.vector.reciprocal(rstd, ssq)
                rstd4 = rstd.rearrange("p (g h) -> p g h", g=GT)
                # fold q_gain into the q rstds
                nc.vector.tensor_mul(rstd4[:, :, 0:NH], rstd4[:, :, 0:NH], qg4)
                # stash k rstd * SCALE for the exp
                nc.scalar.mul(
                    out=rstdk_sb[:, ts(g, GT)],
                    in_=rstd4[:, :, NH : NH + 1].rearrange("p g o -> p (g o)"),
                    mul=SCALE,
                )

                # rope in place on raw q|k (rotation commutes with rms scale)
                q1 = qk_raw[:, :, 0 : HD // 2]
                q2 = qk_raw[:, :, HD // 2 : HD]
                cg = cos_sb[:, ts(g, GT), :, :].rearrange("p g h x -> p (g h) x")
                sg = sin_sb[:, ts(g, GT), :, :].rearrange("p g h x -> p (g h) x")
                t_a = qk_pool.tile([P, nqk, HD // 2], F32, tag="ta")
                t_b = qk_pool.tile([P, nqk, HD // 2], F32, tag="tb")
                t_c = qk_pool.tile([P, nqk, HD // 2], F32, tag="tc")
                t_d = qk_pool.tile([P, nqk, HD // 2], F32, tag="td")
                nc.vector.tensor_mul(t_a, q1, cg)
                nc.vector.tensor_mul(t_b, q2, sg)
                nc.vector.tensor_mul(t_c, q1, sg)
                nc.vector.tensor_mul(t_d, q2, cg)
                nc.vector.tensor_add(q1, t_a, t_b)
                nc.vector.tensor_sub(q2, t_d, t_c)

                # scale q heads by rstd (casting to bf16); copy k unscaled
                # (its rms scale is folded into the exp)
                qk_c = qk_pool.tile([P, nqk, HD], BF16, tag="qkc")
                for j in range(GT):
                    for h in range(NH):
                        i = j * (NH + 1) + h
                        nc.vector.tensor_scalar_mul(
                            out=qk_c[:, i, :],
                            in0=qk_raw[:, i, :],
                            scalar1=rstd[:, i : i + 1],
                        )
                    ik_ = j * (NH + 1) + NH
                    nc.vector.tensor_copy(qk_c[:, ik_, :], qk_raw[:, ik_, :])

                # bf16 transposes: q head-pairs and k
                for j in range(GT):
                    it = g * GT + j
                    i0 = j * (NH + 1)
                    for pr in range(2):
                        trq = tr_ps_pool.tile([P, P], BF16, tag="trq" if tr_ps_pool is not qkv_ps_pool else "s")
                        nc.tensor.transpose(
                            trq, qk_c[:, i0 + 2 * pr : i0 + 2 * pr + 2, :], ident
                        )
                        nc.vector.tensor_copy(qT_sb[:, pr, ts(it, P)], trq)
                    trk = tr_ps_pool.tile([P, P], BF16, tag="trq" if tr_ps_pool is not qkv_ps_pool else "s")
                    nc.tensor.transpose(trk[0:64, :], qk_c[:, i0 + NH, :], ident)
                    nc.vector.tensor_copy(kT_sb[0:64, ts(it, P)], trk[0:64, :])
                    # replicate kT into the upper partition half for the
                    # odd-head row-group score matmuls
                    nc.sync.dma_start(
                        out=kT_sb[64:128, ts(it, P)], in_=kT_sb[0:64, ts(it, P)]
                    )

            den_holder = [None]

            def emit_chunk(c, s_ps_pool, y_ps_pool, den_ps_pool):
                den_holder[0] = den_ps_pool
                """Attention for tq chunk c (needs groups 0..c done)."""
                ntk = (c + 1) * (CW // P)
                y_ps = [
                    y_ps_pool.tile([P, CW], F32, tag="y", name=f"y_c{c}_{pr}")
                    for pr in range(2)
                ]
                den_ps = [
                    den_ps_pool.tile([P, CW], F32, tag="den", name=f"den_c{c}_{pr}")
                    for pr in range(2)
                ]
                for tk in range(ntk):
                    dj = tk - 4 * c  # >= 0 on the diagonal block
                    lo = P * dj if dj >= 0 else 0
                    u = u_pool.tile([P, NH, CW], BF16, tag="u")
                    for pr in range(2):
                        s_ps = s_ps_pool.tile([P, 2, CW], F32, tag="s")
                        for hh in range(2):
                            nc.tensor.matmul(
                                s_ps[:, hh, lo:],
                                lhsT=kT_sb[64 * hh : 64 * (hh + 1), ts(tk, P)],
                                rhs=qT_sb[
                                    64 * hh : 64 * (hh + 1),
                                    pr,
                                    c * CW + lo : (c + 1) * CW,
                                ],
                                start=True,
                                stop=True,
                            )
                        nc.scalar.activation(
                            out=u[:, 2 * pr : 2 * pr + 2, lo:],
                            in_=s_ps[:, :, lo:],
                            func=mybir.ActivationFunctionType.Exp,
                            scale=rstdk_sb[:, tk : tk + 1],
                        )
                        if dj >= 0:
                            # multiplicative causal mask on the diagonal
                            # 128-block of u; runs on DVE so the PE->ACT
                            # exp chain is never blocked on it
                            nc.vector.tensor_mul(
                                u[:, 2 * pr : 2 * pr + 2, lo : lo + P],
                                u[:, 2 * pr : 2 * pr + 2, lo : lo + P],
                                tri2b,
                            )
                    if debug and c == 0 and tk == 0:
                        nc.sync.dma_start(out=dbg_u[:], in_=u)
                    horder = (0, 2, 1, 3) if os.environ.get("KB_HORD", "0") == "1" else (0, 1, 2, 3)
                    for h in horder:
                        pr, hh = divmod(h, 2)
                        # even head -> [V|0], odd -> [0|V]
                        vop = v_sb[:, tk, 64:192] if hh == 0 else v_sb[:, tk, 0:128]
                        nc.tensor.matmul(
                            y_ps[pr][:, lo:],
                            lhsT=vop,
                            rhs=u[:, h, lo:],
                            start=(tk == 0 and hh == 0),
                            stop=(tk == ntk - 1 and hh == 1),
                            skip_group_check=True,
                        )
                        nc.tensor.matmul(
                            den_ps[pr][:, lo:],
                            lhsT=den_lhsT(h),
                            rhs=u[:, h, lo:],
                            start=(tk == 0 and hh == 0),
                            stop=(tk == ntk - 1 and hh == 1),
                            skip_group_check=True,
                        )

                for pr in range(2):
                    dr = r_pool.tile([P, CW], F32, tag="dr")
                    nc.vector.reciprocal(dr, den_ps[pr])
                    if debug:
                        nc.sync.dma_start(out=dbg_dr[c, pr], in_=dr)
                        ystg = r_pool.tile([P, CW], F32, tag="ystg")
                        nc.vector.tensor_copy(ystg, y_ps[pr])
                        nc.sync.dma_start(out=dbg_y[c, pr], in_=ystg)
                    nc.vector.tensor_mul(
                        yT_sb[:, pr, ts(c, CW)], y_ps[pr], dr
                    )

            def emit_proj(c, proj_ps_pool):
                """Output projection for t-tiles 4c..4c+3.  proj_ps_pool may
                be the y pool (tag-shared slots) so this can interleave with
                the next chunk's attention."""
                for j in range(GT):
                    it = c * GT + j
                    ob = ob_pool.tile([P, D], F32, tag="ob")
                    for nh_ in range(2):
                        pj = proj_ps_pool.tile(
                            [P, CW], F32,
                            tag="den" if proj_ps_pool is den_holder[0] else "y",
                            name=f"pj{c}_{j}_{nh_}",
                        )
                        for kt in range(2):
                            nc.tensor.matmul(
                                pj,
                                lhsT=yT_sb[:, kt, ts(it, P)],
                                rhs=wproj_sb[:, kt, ts(nh_, CW)],
                                start=(kt == 0),
                                stop=(kt == 1),
                            )
                        if nh_ == 0:
                            nc.scalar.copy(ob[:, 0:CW], pj)
                        else:
                            nc.vector.tensor_copy(ob[:, CW:D], pj)
                    nc.sync.dma_start(out=out_d[ts(it, P), 0:CW], in_=ob[:, 0:CW])
                    nc.sync.dma_start(out=out_d[ts(it, P), CW:D], in_=ob[:, CW:D])

            order = os.environ.get("KERNEL_ORDER", "phases")
            phases = int(os.environ.get("KERNEL_PHASES", "3"))
            if order == "segments":
                # interleaved emission: group g -> chunk g -> proj g, with
                # per-segment PSUM pools
                for seg in range(NG):
                    with (
                        tc.tile_pool(
                            name=f"qkv_ps{seg}", bufs=1, space="PSUM"
                        ) as qkv_ps_pool,
                        tc.tile_pool(
                            name=f"tr_ps{seg}", bufs=2, space="PSUM"
                        ) as tr_ps_pool,
                    ):
                        emit_group(seg, qkv_ps_pool, tr_ps_pool)
                    with (
                        tc.tile_pool(
                            name=f"s_ps{seg}", bufs=2, space="PSUM"
                        ) as s_ps_pool,
                        tc.tile_pool(
                            name=f"y_ps{seg}", bufs=2, space="PSUM"
                        ) as y_ps_pool,
                        tc.tile_pool(
                            name=f"den_ps{seg}", bufs=2, space="PSUM"
                        ) as den_ps_pool,
                    ):
                        emit_chunk(seg, s_ps_pool, y_ps_pool, den_ps_pool)
                    with tc.tile_pool(
                        name=f"proj_ps{seg}", bufs=2, space="PSUM"
                    ) as proj_ps_pool:
                        emit_proj(seg, proj_ps_pool)
            elif order == "mix":
                # fully interleaved: groups and chunks share the score-tile
                # psum slots; proj shares the den slots.  One set of pools
                # spans the whole kernel so cross-phase overlap is limited
                # only by real data deps and slot contention.
                with (
                    tc.tile_pool(name="s_ps", bufs=2, space="PSUM") as s_ps_pool,
                    tc.tile_pool(name="y_ps", bufs=2, space="PSUM") as y_ps_pool,
                    tc.tile_pool(name="den_ps", bufs=2, space="PSUM") as den_ps_pool,
                ):
                    emit_group(0, s_ps_pool, s_ps_pool, qkv_tag="s")
                    emit_group(1, s_ps_pool, s_ps_pool, qkv_tag="s")
                    for c in range(NCHUNK):
                        emit_chunk(c, s_ps_pool, y_ps_pool, den_ps_pool)
                        if c + 2 < NG:
                            emit_group(c + 2, s_ps_pool, s_ps_pool, qkv_tag="s")
                        emit_proj(c, den_ps_pool)
            else:
                # groups 0-1 up front in their own pools; groups 2-3 are
                # woven into the attention stream on the score-tile slots
                # (chunk c only needs groups <= c, so g2 goes after chunk 0
                # and g3 after chunk 1); proj(c) rides the den slots
                nfront = int(os.environ.get("KERNEL_NFRONT", "4"))
                with (
                    tc.tile_pool(name="qkv_ps", bufs=_b("QKV", 1), space="PSUM") as qkv_ps_pool,
                    tc.tile_pool(name="tr_ps", bufs=_b("TR", 4), space="PSUM") as tr_ps_pool,
                ):
                    for g in range(nfront):
                        emit_group(g, qkv_ps_pool, tr_ps_pool)
                if phases >= 2:
                    with (
                        tc.tile_pool(name="s_ps", bufs=2, space="PSUM") as s_ps_pool,
                        tc.tile_pool(name="y_ps", bufs=2, space="PSUM") as y_ps_pool,
                        tc.tile_pool(
                            name="den_ps", bufs=2, space="PSUM"
                        ) as den_ps_pool,
                    ):
                        for c in range(NCHUNK):
                            emit_chunk(c, s_ps_pool, y_ps_pool, den_ps_pool)
                            if c + nfront < NG:
                                emit_group(
                                    c + nfront, s_ps_pool, s_ps_pool, qkv_tag="s"
                                )
                            if phases >= 3:
                                emit_proj(c, den_ps_pool)

            if debug:
                nc.sync.dma_start(out=dbg_qT[:], in_=qT_sb)
                nc.sync.dma_start(out=dbg_kT[:], in_=kT_sb)
                nc.sync.dma_start(out=dbg_v[:], in_=v_sb)
                nc.sync.dma_start(out=dbg_rk[:], in_=rstdk_sb)
                nc.sync.dma_start(out=dbg_yT[:], in_=yT_sb)

    nc.finalize()
    return nc


_NC_CACHE = {}


def _get_nc(debug=False):
    key = "dbg" if debug else "nc"
    if key not in _NC_CACHE:
        _NC_CACHE[key] = _build_bass(debug=debug)
    return _NC_CACHE[key]


def _make_consts(q_gain_local):
    consts = np.zeros((P, NCONST), dtype=np.float32)
    pi = np.arange(P)
    # additive causal mask for the diagonal block: 0 where tq >= tk (f >= p)
    madd = np.where(np.arange(P)[None, :] >= pi[:, None], 0.0, -1e30).astype(
        np.float32
    )
    consts[:, C_TRI : C_TRI + 128] = madd
    consts[:, C_TRI + 128 : C_TRI + 256] = madd
    consts[:, C_QG : C_QG + 16] = np.tile(
        np.asarray(q_gain_local, np.float32)[None, :], (P, GT)
    )
    consts[:, C_EPS] = EPS
    return consts


def _make_blk():
    blk = np.zeros((P, BLK_W), dtype=ml_dtypes.bfloat16)
    blk[:, 64:128] = 1.0
    blk[:, 256:384] = np.eye(P, dtype=np.float32).astype(ml_dtypes.bfloat16)
    tri = (np.arange(P)[None, :] >= np.arange(P)[:, None]).astype(np.float32)
    blk[:, 384:512] = tri.astype(ml_dtypes.bfloat16)
    blk[:, 512:640] = tri.astype(ml_dtypes.bfloat16)
    return blk


def _rope_tables():
    inv = 1.0 / (
        ROPE_BASE ** (np.arange(0, HD, 2, dtype=np.float32) / HD)
    )
    f = np.arange(T, dtype=np.float32)[:, None] * inv[None, :]
    cos = np.cos(f).astype(np.float32)
    sin = np.sin(f).astype(np.float32)
    # replicate across the 4 q heads + 1 k head (walrus rejects zero-step
    # broadcast APs in TensorTensor, so the broadcast happens host-side)
    cos5 = np.ascontiguousarray(
        np.broadcast_to(cos[:, None, :], (T, NH + 1, HD // 2))
    )
    sin5 = np.ascontiguousarray(
        np.broadcast_to(sin[:, None, :], (T, NH + 1, HD // 2))
    )
    return cos5, sin5


def _make_in_maps(x, Wq, Wk, Wv, Wproj, q_gain):
    x = np.ascontiguousarray(np.asarray(x, np.float32))
    Wq = np.asarray(Wq, np.float32)
    Wk = np.asarray(Wk, np.float32)
    Wv = np.asarray(Wv, np.float32)
    Wproj = np.asarray(Wproj, np.float32)
    q_gain = np.asarray(q_gain, np.float32)
    cos, sin = _rope_tables()
    bf16 = ml_dtypes.bfloat16
    xTs = [np.ascontiguousarray(x[b].T.astype(bf16)) for b in range(B)]
    blk = _make_blk()
    kvw = HKV * HD  # 256 per-core q slice width
    in_maps = []
    for core in range(8):
        b, g = divmod(core, HKV)
        wq = Wq[g * kvw : (g + 1) * kvw]
        wk = Wk[g * HD : (g + 1) * HD]
        wv = Wv[g * HD : (g + 1) * HD]
        wqkvT = np.ascontiguousarray(np.concatenate([wq, wk, wv], 0).T.astype(bf16))
        wprojT = np.ascontiguousarray(
            Wproj[:, g * kvw : (g + 1) * kvw].T.astype(bf16)
        )
        consts = _make_consts(q_gain[g * NH : (g + 1) * NH])
        in_maps.append(
            {
                "xT": xTs[b],
                "wqkvT": wqkvT,
                "wprojT": wprojT,
                "cosT": cos,
                "sinT": sin,
                "consts": consts,
                "blk": blk,
            }
        )
    return in_maps


def run_sharded(inputs, trace=False, debug=False, **kwargs):
    """Run the SPMD kernel; returns (full_output, BassKernelResults)."""
    in_maps = _make_in_maps(**inputs)
    res = run_bass_kernel_spmd(
        _get_nc(debug=debug), in_maps, core_ids=list(range(8)), trace=trace,
        **kwargs
    )
    out = np.zeros((B, T, D), np.float32)
    for core in range(8):
        out[core // HKV] += res.results[core]["outp"]
    return out, res


def kernel(x, Wq, Wk, Wv, Wproj, q_gain):
    out, _ = run_sharded(
        dict(x=x, Wq=Wq, Wk=Wk, Wv=Wv, Wproj=Wproj, q_gain=q_gain)
    )
    return out



# revision 26
# speedup vs baseline: 1.2968x; 1.2968x over previous
"""Causal self-attention (GQA, qk-RMS-norm, RoPE) Trainium2 Bass kernel.

Sharding (8 cores): batch (2) x kv-head-group (4).  Core c handles batch
b = c // 4 and kv head g = c % 4 (with its 4 query heads 4g..4g+3).
Each core computes a (T, D) partial of the output projection (Wproj input
dim is split across the 4 tensor-parallel cores); the host sums the 4
partials per batch element (partials are written fp16; the host
accumulates in f32).

v2 design (everything fp16 device-side; f32 only in PSUM and stats):
  - QKV proj: psum[t_tile, 384] = sum_k xT_tile^T . wqkvT_tile
  - rms stats on raw q/k; the rsqrt runs entirely on DVE (bit-trick seed
    + 2 Newton steps) so the ACT engine only ever needs the Exp table;
    per-head rstd_q * q_gain * SCALE and rstd_k are folded into the fp16
    cast of q/k (gpsimd), so the exp needs no scale operand at all
  - rope applied to raw q/k in fp16 (DVE 2x mode)
  - scores computed transposed S^T[tk, tq] (no softmax max pass needed:
    |s| <= 8 after rms norm), exp straight out of PSUM on ACT,
    multiplicative {0,1} tri-mask on the diagonal 128-block on gpsimd
  - A.V in the y = [tq, d] orientation: one matmul per (head, tq-128,
    tk-tile) with rhs = [V | ones] (65 cols) accumulating y AND the
    softmax denominator in the same psum bank; head-major order because
    PSUM start zeroes the whole 2KB bank lazily.  This replaces the old
    yT-orientation A.V + separate denominator matmuls (~55us less PE)
  - normalization is a per-partition scalar multiply (reciprocal of the
    den column) on DVE, then y head-pairs are PE-transposed to yT
    feeding the output projection
  - kT lives in both 64-partition halves (score matmuls use PE row
    groups for the 2 heads of a pair); replicated by a second PE
    transpose instead of a DMA
  - groups are woven into the chunk stream (chunk c only needs groups
    <= c): qkv psum rides the score-tile slots, transposes ride the
    m-pool slots, so the group-phase DVE/Pool work overlaps chunk
    exp/matmul work
  - DMAs are batched: one xT load per t-tile group, one fp16 output
    store per t-tile.
"""

import os
import sys

import numpy as np

for _p in ("/opt/trn_rl_repo", "/root/.axon_site/_ro/trn_rl_repo"):
    if _p not in sys.path and os.path.isdir(_p):
        sys.path.append(_p)

import concourse.bass as bass
import concourse.bacc as bacc_mod
import concourse.mybir as mybir
import concourse.tile as tile
from concourse.bass import ts
from concourse.bass_utils import run_bass_kernel_spmd

F32 = mybir.dt.float32
F16 = mybir.dt.float16
I32 = mybir.dt.int32

B, T, D = 2, 2048, 1024
H, HKV, HD = 16, 4, 64
NH = H // HKV            # 4 local q heads per core
P = 128
NT = T // P              # 16 t tiles
ND = D // P              # 8 d tiles
GT = 4                   # t-tiles per qkv group
NG = NT // GT            # 4 groups
CW = 512                 # tq chunk width
NCHUNK = T // CW         # 4
QKV_W = NH * HD + 2 * HD  # 384 = q(256) | k(64) | v(64)
NQK = GT * (NH + 1)      # 20 rms/rope lanes per group
EPS = float(np.finfo(np.float32).eps)
SCALE = float(HD) ** -0.5
ROPE_BASE = 10000.0
RSQRT_MAGIC = 0x5F3759DF

# consts layout (f32): [0:20] per-lane rstd fold factor
#   q lanes: q_gain * SCALE * sqrt(HD) = q_gain;  k lanes: sqrt(HD) = 8
# (the DVE rsqrt computes 1/sqrt(ssq + HD*eps) = rstd/sqrt(HD))
NCONST = NQK

# blk layout (fp16): [0:128] identity, [128:640] tri {0,1} mask x4 heads
BLK_W = 640


def _build_bass():
    nc = bacc_mod.Bacc(trn_type="TRN2")

    xT_d = nc.dram_tensor("xT", [D, T], F16, kind="ExternalInput")
    wqkv_d = nc.dram_tensor("wqkvT", [D, QKV_W], F16, kind="ExternalInput")
    wproj_d = nc.dram_tensor("wprojT", [NH * HD, D], F16, kind="ExternalInput")
    # rope tables pre-shuffled host-side to [P, NT, 2, 5, 32] (cos|sin in
    # one tensor, one DMA) so the DMA moves >=512B contiguous runs
    rope_d = nc.dram_tensor(
        "ropeT", [P, 2, NT, NH + 1, HD // 2], F16, kind="ExternalInput"
    )
    # blk carries the f32 qgk consts bitcast into its last 2*NCONST cols
    blk_d = nc.dram_tensor("blk", [P, BLK_W + 2 * NCONST], F16,
                           kind="ExternalInput")
    out_d = nc.dram_tensor("outp", [T, D], F16, kind="ExternalOutput")

    with tile.TileContext(nc) as tc:
        with (
            tc.tile_pool(name="singles", bufs=1) as singles,
            tc.tile_pool(name="xg", bufs=4) as xg_pool,
            tc.tile_pool(name="qk", bufs=2) as qk_pool,
            tc.tile_pool(name="stat", bufs=2) as stat_pool,
            tc.tile_pool(name="u", bufs=16) as u_pool,
            tc.tile_pool(name="r", bufs=2) as r_pool,
            tc.tile_pool(name="ysb", bufs=2) as ysb_pool,
            tc.tile_pool(name="ob", bufs=2) as ob_pool,
            tc.tile_pool(name="s_ps", bufs=2, space="PSUM") as s_pool,
            tc.tile_pool(name="y_ps", bufs=2, space="PSUM") as y_pool,
            tc.tile_pool(name="m_ps", bufs=2, space="PSUM") as m_pool,
        ):
            # ---------------- persistent SBUF ----------------
            wqkv_sb = singles.tile([P, ND, QKV_W], F16)
            wproj_sb = singles.tile([P, 2, D], F16)
            rope_sb = singles.tile([P, 2, NT, NH + 1, HD // 2], F16)
            blk_sb = singles.tile([P, BLK_W + 2 * NCONST], F16)
            magic_sb = singles.tile([P, NQK], I32)
            # pair pr: head 2pr at partitions 0:64, head 2pr+1 at 64:128
            qT_sb = singles.tile([P, 2, T], F16)
            kT_sb = singles.tile([P, T], F16)      # replicated into both halves
            v_sb = singles.tile([P, NT, 66], F16)  # cols 0:64 V, col 64 ones
            yT_sb = singles.tile([P, 2, T], F16)

            # DMA order matters: the transfer device serializes, and the
            # first QKV matmuls need wqkv + xg0 (issued in emit_group(0))
            nc.sync.dma_start(
                out=wqkv_sb, in_=wqkv_d[:].rearrange("(po pi) f -> pi po f", pi=P)
            )
            nc.gpsimd.memset(v_sb, 0.0)
            nc.gpsimd.memset(v_sb[:, :, 64:65], 1.0)
            nc.gpsimd.memset(magic_sb, RSQRT_MAGIC)

            def emit_early_dmas():
                # after xg0: blk (transposes ~12us), rope (rope ~12us)
                nc.sync.dma_start(out=blk_sb, in_=blk_d[:])
                nc.sync.dma_start(out=rope_sb, in_=rope_d[:])

            def emit_late_dmas():
                # wproj is first needed by proj(0), much later
                nc.sync.dma_start(
                    out=wproj_sb,
                    in_=wproj_d[:].rearrange("(po pi) f -> pi po f", pi=P),
                )

            qgk = blk_sb[:, BLK_W : BLK_W + 2 * NCONST].bitcast(F32)
            ident = blk_sb[:, 0:128]
            tri4 = blk_sb[:, 128:640].rearrange("p (j f) -> p j f", j=4)

            # PE p-state warmup: the tensor engine ramps to full clock only
            # after ~3us of continuous execution, and the cost of a matmul is
            # locked at dispatch.  Junk matmuls (no DMA dependency) keep PE
            # busy from t~0.7us until the first xg tile lands, so all real
            # matmuls price at the full clock.
            warm_sb = singles.tile([P, CW], F16)
            nc.gpsimd.memset(warm_sb, 0.0)

            def emit_warmup(n):
                wps = s_pool.tile([P, 2, CW], F32, tag="s", name="warm")
                for i in range(n):
                    nc.tensor.matmul(
                        wps[:, i % 2, :],
                        lhsT=warm_sb[:, 0:128],
                        rhs=warm_sb,
                        start=True,
                        stop=True,
                        skip_group_check=True,
                    )

            xg_tiles = {}

            def emit_xg(g):
                xg_sb = xg_pool.tile([P, ND, GT * P], F16, tag="xg", name=f"xg{g}")
                nc.sync.dma_start(
                    out=xg_sb,
                    in_=xT_d[:, ts(g, GT * P)].rearrange(
                        "(po pi) t -> pi po t", pi=P
                    ),
                )
                xg_tiles[g] = xg_sb

            def emit_group(g):
                """QKV projection + rms stats + rope + fp16 cast (with the
                rstd/q_gain/SCALE folds) + transposes for group g.  QKV psum
                rides the score-tile slots; transposes ride the m slots."""
                xg_sb = xg_tiles[g]
                qk_raw = qk_pool.tile([P, NQK, HD], F16, tag="qkraw")
                sq = qk_pool.tile([P, NQK, HD], F16, tag="sq")
                for half in range(2):
                    qkv_ps = s_pool.tile([P, 2, 512], F32, tag="s",
                                         name=f"qkv{g}_{half}")
                    for jj in range(2):
                        j = 2 * half + jj
                        for ik in range(ND):
                            nc.tensor.matmul(
                                qkv_ps[:, jj, 0:QKV_W],
                                lhsT=xg_sb[:, ik, ts(j, P)],
                                rhs=wqkv_sb[:, ik, :],
                                start=(ik == 0),
                                stop=(ik == ND - 1),
                            )
                    h0 = 2 * half * (NH + 1)
                    nc.vector.tensor_copy(
                        qk_raw[:, h0 : h0 + 2 * (NH + 1), :].rearrange(
                            "p n x -> p (n x)"
                        ),
                        qkv_ps[:, :, 0 : (NH + 1) * HD],
                    )
                    # squares on ACT (Square shares the Exp table) straight
                    # from PSUM: takes stats off the rope critical path
                    nc.scalar.square(
                        sq[:, h0 : h0 + 2 * (NH + 1), :].rearrange(
                            "p n x -> p (n x)"
                        ),
                        qkv_ps[:, :, 0 : (NH + 1) * HD],
                    )
                    nc.vector.tensor_copy(
                        v_sb[:, ts(2 * g + half, 2), 0:64],
                        qkv_ps[:, :, 320:384],
                    )

                ssq = stat_pool.tile([P, NQK], F32, tag="ssq")
                nc.vector.reduce_sum(ssq, sq, axis=mybir.AxisListType.X)
                nc.vector.tensor_scalar_add(
                    out=ssq, in0=ssq, scalar1=float(HD * EPS)
                )
                # rstd/sqrt(HD) = rsqrt(ssq): bit-trick seed + 2 Newton steps
                # (keeps ACT pinned to the Exp table: no table reloads)
                rstd = stat_pool.tile([P, NQK], F32, tag="rstd")
                nt_ = stat_pool.tile([P, NQK], F32, tag="nt")
                rstd_i = rstd[:, :].bitcast(I32)
                nc.vector.tensor_scalar(
                    out=rstd_i,
                    in0=ssq[:, :].bitcast(I32),
                    scalar1=1,
                    scalar2=None,
                    op0=mybir.AluOpType.logical_shift_right,
                )
                nc.vector.tensor_sub(rstd_i, magic_sb, rstd_i)
                for _ in range(2):
                    nc.vector.tensor_mul(nt_, rstd, rstd)
                    nc.vector.tensor_mul(nt_, nt_, ssq)
                    nc.vector.tensor_scalar(
                        out=nt_,
                        in0=nt_,
                        scalar1=-0.5,
                        scalar2=1.5,
                        op0=mybir.AluOpType.mult,
                        op1=mybir.AluOpType.add,
                    )
                    nc.vector.tensor_mul(rstd, rstd, nt_)
                # fold q_gain (q lanes) / sqrt(HD) restore (all lanes)
                nc.vector.tensor_mul(rstd, rstd, qgk)

                # rope in place on raw q|k (rotation commutes with rms scale)
                q1 = qk_raw[:, :, 0 : HD // 2]
                q2 = qk_raw[:, :, HD // 2 : HD]
                cg = rope_sb[:, 0:1, ts(g, GT), :, :].rearrange(
                    "p o g h x -> p (o g h) x"
                )
                sg = rope_sb[:, 1:2, ts(g, GT), :, :].rearrange(
                    "p o g h x -> p (o g h) x"
                )
                t_a = qk_pool.tile([P, NQK, HD // 2], F16, tag="ta")
                t_b = qk_pool.tile([P, NQK, HD // 2], F16, tag="tb")
                t_c = qk_pool.tile([P, NQK, HD // 2], F16, tag="tc")
                t_d = qk_pool.tile([P, NQK, HD // 2], F16, tag="td")
                nc.vector.tensor_mul(t_a, q1, cg)
                nc.vector.tensor_mul(t_b, q2, sg)
                nc.vector.tensor_mul(t_c, q1, sg)
                nc.vector.tensor_mul(t_d, q2, cg)
                nc.vector.tensor_add(q1, t_a, t_b)
                nc.vector.tensor_sub(q2, t_d, t_c)

                # scale q heads by rstd*gain*SCALE and k by rstd_k (fp16);
                # first half on DVE (shortest latency to the first
                # transposes), second half offloaded to gpsimd
                qk_c = qk_pool.tile([P, NQK, HD], F16, tag="qkc")
                for i in range(NQK):
                    eng = nc.vector if i < NQK // 2 else nc.gpsimd
                    eng.tensor_scalar_mul(
                        out=qk_c[:, i, :],
                        in0=qk_raw[:, i, :],
                        scalar1=rstd[:, i : i + 1],
                    )

                # fp16 transposes: q head-pairs; k twice (both 64-part halves)
                for j in range(GT):
                    it = g * GT + j
                    i0 = j * (NH + 1)
                    for pr in range(2):
                        trq = m_pool.tile([P, 2, CW], F16, tag="m",
                                          name=f"trq{it}_{pr}")
                        nc.tensor.transpose(
                            trq[:, 0, 0:128],
                            qk_c[:, i0 + 2 * pr : i0 + 2 * pr + 2, :],
                            ident,
                        )
                        nc.vector.tensor_copy(
                            qT_sb[:, pr, ts(it, P)], trq[:, 0, 0:128]
                        )
                    trk = m_pool.tile([P, 2, CW], F16, tag="m", name=f"trk{it}")
                    nc.tensor.transpose(
                        trk[0:64, 0, 0:128], qk_c[:, i0 + NH, :], ident
                    )
                    nc.tensor.transpose(
                        trk[64:128, 0, 0:128], qk_c[:, i0 + NH, :], ident
                    )
                    nc.vector.tensor_copy(kT_sb[:, ts(it, P)], trk[:, 0, 0:128])

            def emit_scores(c, tk, u_tiles):
                dj = tk - 4 * c  # >= 0 on the diagonal tiles
                lo = P * dj if dj >= 0 else 0
                u = u_pool.tile([P, NH, CW], F16, tag="u",
                                name=f"u_c{c}_{tk}")
                for pr in range(2):
                    s_ps = s_pool.tile([P, 2, CW], F32, tag="s")
                    for hh in range(2):
                        nc.tensor.matmul(
                            s_ps[:, hh, lo:],
                            lhsT=kT_sb[64 * hh : 64 * (hh + 1), ts(tk, P)],
                            rhs=qT_sb[
                                64 * hh : 64 * (hh + 1),
                                pr,
                                c * CW + lo : (c + 1) * CW,
                            ],
                            start=True,
                            stop=True,
                        )
                    nc.scalar.activation(
                        out=u[:, 2 * pr : 2 * pr + 2, lo:],
                        in_=s_ps[:, :, lo:],
                        func=mybir.ActivationFunctionType.Exp,
                    )
                if dj >= 0:
                    # multiplicative causal mask on the diagonal 128-block
                    nc.gpsimd.tensor_mul(
                        u[:, :, lo : lo + P], u[:, :, lo : lo + P], tri4
                    )
                u_tiles.append(u)

            def emit_av(c, s, u_tiles):
                """A.V + normalize + yT transpose for sub-chunk s (t-tile
                it = 4c+s); needs u tiles 0..4c+s."""
                it = c * GT + s
                nk = 4 * c + s + 1
                # y tile padded to a full 2KB psum bank: PSUM start zeroing
                # is lazy per whole bank
                y_ps = y_pool.tile([P, NH, 128], F32, tag="y",
                                   name=f"y_c{c}_{s}")
                # head-major: each head's full accumulation chain before the
                # next head's start
                for h in range(NH):
                    for tk in range(nk):
                        nc.tensor.matmul(
                            y_ps[:, h, 0:65],
                            lhsT=u_tiles[tk][:, h, ts(s, P)],
                            rhs=v_sb[:, tk, 0:65],
                            start=(tk == 0),
                            stop=(tk == nk - 1),
                            skip_group_check=True,
                        )
                dr = r_pool.tile([P, NH], F32, tag="dr")
                nc.vector.reciprocal(dr, y_ps[:, :, 64:65])
                y_sb = ysb_pool.tile([P, NH, HD], F16, tag="ysb")
                for h in range(NH):
                    nc.vector.tensor_scalar_mul(
                        out=y_sb[:, h, :],
                        in0=y_ps[:, h, 0:64],
                        scalar1=dr[:, h : h + 1],
                    )
                for pr in range(2):
                    try_ = m_pool.tile([P, 2, CW], F16, tag="m",
                                       name=f"try_{it}_{pr}")
                    nc.tensor.transpose(
                        try_[:, 0, 0:128],
                        y_sb[:, 2 * pr : 2 * pr + 2, :],
                        ident,
                    )
                    nc.vector.tensor_copy(
                        yT_sb[:, pr, ts(it, P)], try_[:, 0, 0:128]
                    )

            def emit_proj_tile(it):
                """Output projection for t-tile it -> fp16 partial."""
                ob = ob_pool.tile([P, D], F16, tag="ob")
                for nh_ in range(2):
                    pj = m_pool.tile([P, CW], F32, tag="m",
                                     name=f"pj{it}_{nh_}")
                    for kt in range(2):
                        nc.tensor.matmul(
                            pj,
                            lhsT=yT_sb[:, kt, ts(it, P)],
                            rhs=wproj_sb[:, kt, ts(nh_, CW)],
                            start=(kt == 0),
                            stop=(kt == 1),
                        )
                    nc.vector.tensor_copy(ob[:, ts(nh_, CW)], pj)
                nc.sync.dma_start(out=out_d[ts(it, P), :], in_=ob)

            def emit_chunk(c):
                """Attention for tq chunk c (needs groups 0..c done).  The
                AV/normalize/proj work for sub-chunk s is woven in right
                after exp(tk=4c+s) so PE never waits for the whole exp
                stream, and the previous t-tile's projection rides along."""
                ntk = (c + 1) * (CW // P)
                u_tiles = []
                for tk in range(4 * c):
                    emit_scores(c, tk, u_tiles)
                for s in range(GT):
                    emit_scores(c, 4 * c + s, u_tiles)
                    emit_av(c, s, u_tiles)
                    it = c * GT + s
                    if it > 0:
                        emit_proj_tile(it - 1)

            emit_warmup(16)
            emit_xg(0)
            emit_early_dmas()
            emit_xg(1)
            emit_group(0)
            emit_xg(2)
            emit_xg(3)
            emit_group(1)
            emit_late_dmas()
            emit_chunk(0)
            emit_group(2)
            emit_chunk(1)
            emit_group(3)
            emit_chunk(2)
            emit_chunk(3)
            emit_proj_tile(NT - 1)

    nc.finalize()
    return nc


_NC_CACHE = {}


def _get_nc():
    if "nc" not in _NC_CACHE:
        _NC_CACHE["nc"] = _build_bass()
    return _NC_CACHE["nc"]


def _make_blk(q_gain_local):
    blk = np.zeros((P, BLK_W + 2 * NCONST), dtype=np.float16)
    blk[:, 0:128] = np.eye(P, dtype=np.float32)
    tri = (np.arange(P)[None, :] >= np.arange(P)[:, None]).astype(np.float32)
    for j in range(4):
        blk[:, 128 + 128 * j : 256 + 128 * j] = tri
    # per-lane rstd fold factor (f32, bitcast into the f16 tail):
    # q lanes: q_gain * SCALE * sqrt(HD) = q_gain;  k lanes: sqrt(HD)
    lane = np.empty((NQK,), np.float32)
    for j in range(GT):
        lane[j * (NH + 1) : j * (NH + 1) + NH] = np.asarray(
            q_gain_local, np.float32
        )
        lane[j * (NH + 1) + NH] = np.sqrt(HD)
    qgk = np.broadcast_to(lane[None, :], (P, NQK)).astype(np.float32)
    blk[:, BLK_W : BLK_W + 2 * NCONST] = np.ascontiguousarray(qgk).view(
        np.float16
    )
    return blk


def _rope_tables():
    inv = 1.0 / (ROPE_BASE ** (np.arange(0, HD, 2, dtype=np.float32) / HD))
    f = np.arange(T, dtype=np.float32)[:, None] * inv[None, :]
    # replicate across the 4 q heads + 1 k head (no zero-stride broadcast
    # APs in TensorTensor), pre-shuffled to the [P, NT, 2, 5, 32] SBUF
    # layout (cos|sin in one tensor) so the DMA is one contiguous run per
    # partition
    def shuf(a):
        a5 = np.broadcast_to(a[:, None, :], (T, NH + 1, HD // 2))
        # [T=(nt p), h, f] -> [p, nt, h, f]
        return a5.reshape(NT, P, NH + 1, HD // 2).transpose(1, 0, 2, 3)

    rope = np.stack([shuf(np.cos(f)), shuf(np.sin(f))], axis=1)
    return np.ascontiguousarray(rope).astype(np.float16)


def _make_in_maps(x, Wq, Wk, Wv, Wproj, q_gain):
    x = np.ascontiguousarray(np.asarray(x, np.float32))
    Wq = np.asarray(Wq, np.float32)
    Wk = np.asarray(Wk, np.float32)
    Wv = np.asarray(Wv, np.float32)
    Wproj = np.asarray(Wproj, np.float32)
    q_gain = np.asarray(q_gain, np.float32)
    rope = _rope_tables()
    xTs = [np.ascontiguousarray(x[b].T.astype(np.float16)) for b in range(B)]
    kvw = NH * HD  # 256 per-core q slice width
    in_maps = []
    for core in range(8):
        b, g = divmod(core, HKV)
        wq = Wq[g * kvw : (g + 1) * kvw]
        wk = Wk[g * HD : (g + 1) * HD]
        wv = Wv[g * HD : (g + 1) * HD]
        wqkvT = np.ascontiguousarray(
            np.concatenate([wq, wk, wv], 0).T.astype(np.float16)
        )
        wprojT = np.ascontiguousarray(
            Wproj[:, g * kvw : (g + 1) * kvw].T.astype(np.float16)
        )
        in_maps.append(
            {
                "xT": xTs[b],
                "wqkvT": wqkvT,
                "wprojT": wprojT,
                "ropeT": rope,
                "blk": _make_blk(q_gain[g * NH : (g + 1) * NH]),
            }
        )
    return in_maps


def run_sharded(inputs, trace=False, **kwargs):
    """Run the SPMD kernel; returns (full_output, BassKernelResults)."""
    in_maps = _make_in_maps(**inputs)
    res = run_bass_kernel_spmd(
        _get_nc(), in_maps, core_ids=list(range(8)), trace=trace, **kwargs
    )
    out = np.zeros((B, T, D), np.float32)
    for core in range(8):
        out[core // HKV] += res.results[core]["outp"].astype(np.float32)
    return out, res


def kernel(x, Wq, Wk, Wv, Wproj, q_gain):
    out, _ = run_sharded(
        dict(x=x, Wq=Wq, Wk=Wk, Wv=Wv, Wproj=Wproj, q_gain=q_gain)
    )
    return out
